# revision 1
# baseline (speedup 1.0000x reference)
"""EventEmbeddingModel Trainium2 kernel.

kernel(**inputs) takes the FULL (unsharded) inputs and returns the full
[B, D] float32 output.  Internally it is data-parallel over the batch:
each of the 8 NeuronCores gets a contiguous 1024-row slice of the batch;
the embedding table and LinearQ weights are replicated.

Per-core bass program (built once, cached):
  For each chunk of 128 batch rows:
   - load ids/times/lens; compute decay weights w[b,l] = exp(t_l - ct) * mask
     on the scalar/vector engines; the empty-history fallback is folded into
     slot 0 (idx <- ent_id, w <- 1 when hist_len == 0)
   - shuffle w and hist_ids into "pair layout" via a PE transpose: partition k
     of column J holds (b=2J, slot=k) for k<64 and (b=2J+1, slot=k-64)
   - indirect-DMA gather of emb rows, one [128,1] index column per DMA
   - per pair J: two matmuls (d halves) with the gathered block stationary and
     a [128,2] block-diagonal weight column moving -> accumulates his^T in
     PSUM laid out [d, b]
   - final linear y = his @ W^T + b: his^T is exactly the lhsT the PE needs,
     so two accumulating matmuls against W^T halves, bias added via a
     PE-broadcast bias tile, then DMA out.
"""
import sys

import numpy as np

if "/opt/trn_rl_repo" not in sys.path:
    sys.path.insert(0, "/opt/trn_rl_repo")

B, L, V, D = 8192, 64, 100000, 256
N_CORES = 8
BL = B // N_CORES
P = 128
NCHUNK = BL // P
NPAIR = L


def build_nc(debug=False, reps=1):
    import concourse.bass as bass
    import concourse.tile as tile
    from concourse import bacc, mybir
    from concourse.masks import make_identity

    f32, i32 = mybir.dt.float32, mybir.dt.int32
    op = mybir.AluOpType
    act = mybir.ActivationFunctionType

    nc = bacc.Bacc("TRN2", target_bir_lowering=False, debug=debug,
                   num_devices=N_CORES)

    ent = nc.dram_tensor("ent_ids", [BL], i32, kind="ExternalInput").ap()
    ct_d = nc.dram_tensor("current_time", [BL], f32, kind="ExternalInput").ap()
    hid = nc.dram_tensor("hist_ids", [BL, L], i32, kind="ExternalInput").ap()
    ht_d = nc.dram_tensor("hist_times", [BL, L], f32, kind="ExternalInput").ap()
    hl_d = nc.dram_tensor("hist_len", [BL], i32, kind="ExternalInput").ap()
    emb = nc.dram_tensor("emb", [V, D], f32, kind="ExternalInput").ap()
    wt_d = nc.dram_tensor("WT", [D, D], f32, kind="ExternalInput").ap()
    b_d = nc.dram_tensor("bvec", [D], f32, kind="ExternalInput").ap()
    y_d = nc.dram_tensor("y", [BL, D], f32, kind="ExternalOutput").ap()

    with tile.TileContext(nc) as tc:
        with tc.tile_pool(name="const", bufs=1) as cpool, \
             tc.tile_pool(name="io", bufs=2) as iop, \
             tc.tile_pool(name="stage", bufs=2) as stp, \
             tc.tile_pool(name="gather", bufs=8) as gp, \
             tc.tile_pool(name="outp", bufs=2) as outp, \
             tc.tile_pool(name="pt", bufs=1, space="PSUM") as pt, \
             tc.tile_pool(name="phis", bufs=2, space="PSUM") as phis, \
             tc.tile_pool(name="py", bufs=2, space="PSUM") as py:

            # ---- constants ----
            ident = cpool.tile([P, P], f32)
            make_identity(nc, ident[:])

            iota64_i = cpool.tile([P, L], i32)
            nc.gpsimd.iota(iota64_i[:], pattern=[[1, L]], base=0,
                           channel_multiplier=0)
            iota64_f = cpool.tile([P, L], f32)
            nc.vector.tensor_copy(iota64_f[:], iota64_i[:])

            iotap_i = cpool.tile([P, 1], i32)
            nc.gpsimd.iota(iotap_i[:], pattern=[[0, 1]], base=0,
                           channel_multiplier=1)
            iotap_f = cpool.tile([P, 1], f32)
            nc.vector.tensor_copy(iotap_f[:], iotap_i[:])

            halfmask = cpool.tile([P, 2], f32)
            nc.vector.tensor_scalar(halfmask[:, 0:1], iotap_f[:], 64.0, None,
                                    op.is_lt)
            nc.vector.tensor_scalar(halfmask[:, 1:2], iotap_f[:], 63.0, None,
                                    op.is_gt)

            wt0 = cpool.tile([P, D], f32)
            wt1 = cpool.tile([P, D], f32)
            nc.sync.dma_start(out=wt0[:], in_=wt_d[0:P, :])
            nc.sync.dma_start(out=wt1[:], in_=wt_d[P:D, :])

            bias_row = cpool.tile([1, D], f32)
            nc.sync.dma_start(out=bias_row[:], in_=b_d[None, :])
            ones_row = cpool.tile([1, P], f32)
            nc.vector.memset(ones_row[:], 1.0)
            bias_ps = pt.tile([P, D], f32, tag="tw")
            nc.tensor.matmul(out=bias_ps[:], lhsT=ones_row[:], rhs=bias_row[:],
                             start=True, stop=True)
            bias_t = cpool.tile([P, D], f32)
            nc.vector.tensor_copy(bias_t[:], bias_ps[:])

            # ---- per chunk (reps > 1 only for benchmarking) ----
            for _rep in range(reps):
                for c in range(NCHUNK):
                    r0, r1 = c * P, (c + 1) * P

                    idx_nat = iop.tile([P, L], i32)
                    nc.sync.dma_start(out=idx_nat[:], in_=hid[r0:r1, :])
                    ht = iop.tile([P, L], f32)
                    nc.sync.dma_start(out=ht[:], in_=ht_d[r0:r1, :])
                    ct = iop.tile([P, 1], f32)
                    nc.sync.dma_start(out=ct[:], in_=ct_d[r0:r1, None])
                    hl_i = iop.tile([P, 1], i32)
                    nc.sync.dma_start(out=hl_i[:], in_=hl_d[r0:r1, None])
                    eid = iop.tile([P, 1], i32)
                    nc.sync.dma_start(out=eid[:], in_=ent[r0:r1, None])

                    nct = stp.tile([P, 1], f32)
                    nc.vector.tensor_scalar_mul(nct[:], ct[:], -1.0)
                    hl_f = stp.tile([P, 1], f32)
                    nc.vector.tensor_copy(hl_f[:], hl_i[:])

                    # weights in natural layout, duplicated along free dim
                    wdup = stp.tile([P, 2 * L], f32)
                    nc.scalar.activation(out=wdup[:, 0:L], in_=ht[:],
                                         func=act.Exp, bias=nct[:], scale=1.0)
                    mask = stp.tile([P, L], f32)
                    nc.vector.tensor_scalar(mask[:], iota64_f[:], hl_f[:],
                                            None, op.is_lt)
                    nc.vector.tensor_tensor(out=wdup[:, 0:L], in0=wdup[:, 0:L],
                                            in1=mask[:], op=op.mult)
                    m_f = stp.tile([P, 1], f32)
                    nc.vector.tensor_scalar(m_f[:], hl_f[:], 0.0, None,
                                            op.is_equal)
                    nc.vector.tensor_tensor(out=wdup[:, 0:1],
                                            in0=wdup[:, 0:1], in1=m_f[:],
                                            op=op.add)
                    nc.vector.tensor_copy(wdup[:, L:2 * L], wdup[:, 0:L])

                    # fallback id into slot 0 where hist empty
                    m_i = stp.tile([P, 1], i32)
                    nc.vector.tensor_scalar(m_i[:], hl_i[:], 0, None,
                                            op.is_equal)
                    nc.vector.copy_predicated(out=idx_nat[:, 0:1], mask=m_i[:],
                                              data=eid[:])

                    idxdup = stp.tile([P, 2 * L], f32)
                    nc.vector.tensor_copy(idxdup[:, 0:L], idx_nat[:])
                    nc.vector.tensor_copy(idxdup[:, L:2 * L], idx_nat[:])

                    # transpose to pair layout
                    t_w = pt.tile([P, P], f32, tag="tw")
                    nc.tensor.transpose(out=t_w[:], in_=wdup[:],
                                        identity=ident[:])
                    t_i = pt.tile([P, P], f32, tag="ti")
                    nc.tensor.transpose(out=t_i[:], in_=idxdup[:],
                                        identity=ident[:])

                    w_shuf = stp.tile([P, L], f32)
                    nc.vector.tensor_copy(w_shuf[0:64, :], t_w[0:64, 0:P:2])
                    nc.vector.tensor_copy(w_shuf[64:P, :], t_w[64:P, 1:P:2])
                    idx_shuf_f = stp.tile([P, L], f32)
                    nc.vector.tensor_copy(idx_shuf_f[0:64, :],
                                          t_i[0:64, 0:P:2])
                    nc.vector.tensor_copy(idx_shuf_f[64:P, :],
                                          t_i[64:P, 1:P:2])
                    idx_shuf = stp.tile([P, L], i32)
                    nc.vector.tensor_copy(idx_shuf[:], idx_shuf_f[:])

                    # rhs_full[k, 2J+n] = w_shuf[k, J] * halfmask[k, n]
                    rhs_full = stp.tile([P, 2 * L], f32)
                    nc.vector.tensor_tensor(
                        out=rhs_full[:].rearrange("p (j n) -> p j n", n=2),
                        in0=w_shuf[:, :, None].to_broadcast([P, L, 2]),
                        in1=halfmask[:, None, :].to_broadcast([P, L, 2]),
                        op=op.mult)

                    hisT0 = phis.tile([P, P], f32)
                    hisT1 = phis.tile([P, P], f32)

                    for J in range(NPAIR):
                        g = gp.tile([P, D], f32, tag="g")
                        nc.gpsimd.indirect_dma_start(
                            out=g[:], out_offset=None, in_=emb[:],
                            in_offset=bass.IndirectOffsetOnAxis(
                                ap=idx_shuf[:, J:J + 1], axis=0))
                        nc.tensor.matmul(
                            out=hisT0[:, 2 * J:2 * J + 2], lhsT=g[:, 0:P],
                            rhs=rhs_full[:, 2 * J:2 * J + 2],
                            start=True, stop=True)
                        nc.tensor.matmul(
                            out=hisT1[:, 2 * J:2 * J + 2], lhsT=g[:, P:D],
                            rhs=rhs_full[:, 2 * J:2 * J + 2],
                            start=True, stop=True)

                    hisT0_sb = outp.tile([P, P], f32)
                    nc.vector.tensor_copy(hisT0_sb[:], hisT0[:])
                    hisT1_sb = outp.tile([P, P], f32)
                    nc.vector.tensor_copy(hisT1_sb[:], hisT1[:])

                    y_ps = py.tile([P, D], f32)
                    nc.tensor.matmul(out=y_ps[:], lhsT=hisT0_sb[:], rhs=wt0[:],
                                     start=True, stop=False)
                    nc.tensor.matmul(out=y_ps[:], lhsT=hisT1_sb[:], rhs=wt1[:],
                                     start=False, stop=True)

                    y_sb = outp.tile([P, D], f32)
                    nc.vector.tensor_tensor(out=y_sb[:], in0=y_ps[:],
                                            in1=bias_t[:], op=op.add)
                    nc.sync.dma_start(out=y_d[r0:r1, :], in_=y_sb[:])

    nc.compile()
    return nc


_NC_CACHE = {}


def _get_nc():
    if "nc" not in _NC_CACHE:
        _NC_CACHE["nc"] = build_nc()
    return _NC_CACHE["nc"]


def make_in_maps(ent_ids, current_time, hist_ids, hist_times, hist_len,
                 emb, W, b):
    ent_ids = np.ascontiguousarray(np.asarray(ent_ids, dtype=np.int32))
    current_time = np.ascontiguousarray(np.asarray(current_time, np.float32))
    hist_ids = np.ascontiguousarray(np.asarray(hist_ids, dtype=np.int32))
    hist_times = np.ascontiguousarray(np.asarray(hist_times, np.float32))
    hist_len = np.ascontiguousarray(np.asarray(hist_len, dtype=np.int32))
    emb = np.ascontiguousarray(np.asarray(emb, dtype=np.float32))
    WT = np.ascontiguousarray(np.asarray(W, dtype=np.float32).T)
    b = np.ascontiguousarray(np.asarray(b, dtype=np.float32))

    in_maps = []
    for c in range(N_CORES):
        s = slice(c * BL, (c + 1) * BL)
        in_maps.append({
            "ent_ids": ent_ids[s], "current_time": current_time[s],
            "hist_ids": hist_ids[s], "hist_times": hist_times[s],
            "hist_len": hist_len[s], "emb": emb, "WT": WT, "bvec": b,
        })
    return in_maps


def kernel(ent_ids, current_time, hist_ids, hist_times, hist_len, emb, W, b):
    from concourse.bass_utils import run_bass_kernel_spmd

    nc = _get_nc()
    in_maps = make_in_maps(ent_ids, current_time, hist_ids, hist_times,
                           hist_len, emb, W, b)
    res = run_bass_kernel_spmd(nc, in_maps, list(range(N_CORES)))
    return np.concatenate([res.results[c]["y"] for c in range(N_CORES)],
                          axis=0)
